# revision 3
# baseline (speedup 1.0000x reference)
"""CausalGateUnit Trainium2 kernel.

Math (see reference):
  p_x = q @ W_x + b_x (x in {pre,haz});  gates = sigmoid(q @ W_gate + b_gate)
  sim_x = (p_x @ k^T)/sqrt(D), strictly-causal masked (j < i)
  score_x[i] = max_{j<i} sim_x[i,j]   (0 for i==0, fixed up on host)
  out = relu([g_pre, score_pre, g_haz, score_haz] @ W_s1 + b_s1) @ W_s2 + b_s2

Sharding over 8 cores: core = (b, r) with b = core//4, r = core%4.
Core (b, r) owns row tiles t = 4g + r (g = 0..7) of batch b -- 1024 rows.
Slot g computes score chunks over columns [0, 512*(g+1)); every core runs an
identical instruction stream (the 144-subtile schedule is optimal for a
common stream).  The strictly-causal boundary inside the diagonal 512-chunk
is applied by a DVE tensor_add of a per-core constant tile Cm (0 where
j < i, -16384 elsewhere) -- PSUM+SBUF operands, so it runs on the vector
engine and keeps the PE free.

Everything runs in fp16 (same PE speed as bf16, 4 extra mantissa bits).
fp8/DoubleRow was measured at 1.0 cyc/col on this stack (no double pump), so
it is not used.  Input DMAs are split into ~128 KB pieces so they spread
across DMA queues (a single 0.5 MB DMA rides one queue at ~45 GB/s and
stalls phase A).  MLP emission is split into two stages (h1 matmuls+relu,
then h@Ws2) with slot lag so the PE<->ACT ping-pong pipelines across slots
instead of serializing in the tail.
"""

import sys

for _p in ("/opt/trn_rl_repo",):
    if _p not in sys.path:
        sys.path.insert(0, _p)

import numpy as np

B, S, D = 2, 4096, 512
NCORES = 8
P = 128          # partitions / row-tile size
NSLOT = 8        # row tiles per core
ROWS = NSLOT * P  # 1024 rows per core
D1 = 256         # MLP hidden
CHUNK = 512      # score column chunk
KT = D // P      # 4 contraction tiles
NEG = -16384.0   # causal-mask fill (finite in fp16; row 0 fixed on host)

# consts tile free-dim layout (fp16)
ONES_O = 0      # [p0, 0:1024] ones
WS1_O = 1024    # [p0:5, 1024:1280] W_s1 augmented with b_s1
BP_O = 1280     # [p0, 1280:1792] b_pre / sqrt(D)
BH_O = 1792     # [p0, 1792:2304] b_haz / sqrt(D)
BS2_O = 2304    # [p0, 2304:2816] b_s2
BG_O = 2816     # [p0, 2816:2818] b_gate
CONSTW = 2818

_PROGRAM_CACHE = {}


def _build_program(with_bias=True):
    import concourse.bacc as bacc
    import concourse.mybir as mybir
    import concourse.tile as tile

    f32 = mybir.dt.float32
    f16 = mybir.dt.float16
    AX = mybir.AxisListType
    MAX = mybir.AluOpType.max
    ADD = mybir.AluOpType.add
    ACT = mybir.ActivationFunctionType

    nc = bacc.Bacc()

    qT_d = nc.declare_dram_parameter("qT", [D, ROWS], f16, isOutput=False)
    kT_d = nc.declare_dram_parameter("kT", [D, S], f16, isOutput=False)
    Wp_d = nc.declare_dram_parameter("Wp", [D, D], f16, isOutput=False)
    Wh_d = nc.declare_dram_parameter("Wh", [D, D], f16, isOutput=False)
    Wg_d = nc.declare_dram_parameter("Wg", [D, 2], f16, isOutput=False)
    Ws2_d = nc.declare_dram_parameter("Ws2", [D1, D], f16, isOutput=False)
    cn_d = nc.declare_dram_parameter("consts", [P, CONSTW], f16, isOutput=False)
    cm_d = nc.declare_dram_parameter("cm", [P, CHUNK], f16, isOutput=False)
    out_d = nc.declare_dram_parameter("out", [ROWS, D], f32, isOutput=True)

    with tile.TileContext(nc) as tc:
        with (
            tc.tile_pool(name="const", bufs=1) as const,
            tc.tile_pool(name="scpart", bufs=4) as spool,
            tc.tile_pool(name="scfin", bufs=4) as fpool,
            tc.tile_pool(name="outs", bufs=3) as opool,
        ):
            kT_sb = const.tile([P, KT, S], f16)
            qT_sb = const.tile([P, KT, ROWS], f16)
            Wp_sb = const.tile([P, KT, D], f16)
            Wh_sb = const.tile([P, KT, D], f16)
            Wg_sb = const.tile([P, KT, 2], f16)
            Ws2_sb = const.tile([P, 2, D], f16)
            consts_sb = const.tile([P, CONSTW], f16)
            Cm_sb = const.tile([P, CHUNK], f16)
            pTp_sb = const.tile([P, KT, ROWS], f16)
            pTh_sb = const.tile([P, KT, ROWS], f16)
            h1T_sb = const.tile([P, 2, ROWS], f16)
            rsT = const.tile([5, ROWS], f16)
            ones = consts_sb[0:1, ONES_O : ONES_O + ROWS]
            Ws1_sb = consts_sb[0:5, WS1_O : WS1_O + D1]
            bp_sb = consts_sb[0:1, BP_O : BP_O + D]
            bh_sb = consts_sb[0:1, BH_O : BH_O + D]
            bs2_sb = consts_sb[0:1, BS2_O : BS2_O + D]
            bg_sb = consts_sb[0:1, BG_O : BG_O + 2]

            # --- constant loads ---
            # split into ~128-256 KB pieces: one big DMA rides a single queue
            # (~45 GB/s) -- pieces spread across queues and land much sooner.
            # qT rows 0:512 + Wp first: phase A starts as soon as they land.
            qT_r = qT_d[:, :].rearrange("(t p) n -> p t n", p=P)
            for c in range(4):
                cs = slice(c * 128, (c + 1) * 128)
                nc.sync.dma_start(out=qT_sb[:, :, cs], in_=qT_r[:, :, cs])
            Wp_r = Wp_d[:, :].rearrange("(t p) n -> p t n", p=P)
            for c in range(4):
                cs = slice(c * P, (c + 1) * P)
                nc.sync.dma_start(out=Wp_sb[:, :, cs], in_=Wp_r[:, :, cs])
            for c in range(4, 8):
                cs = slice(c * 128, (c + 1) * 128)
                nc.sync.dma_start(out=qT_sb[:, :, cs], in_=qT_r[:, :, cs])
            Wh_r = Wh_d[:, :].rearrange("(t p) n -> p t n", p=P)
            for c in range(4):
                cs = slice(c * P, (c + 1) * P)
                nc.sync.dma_start(out=Wh_sb[:, :, cs], in_=Wh_r[:, :, cs])
            nc.sync.dma_start(out=consts_sb, in_=cn_d[:, :])
            nc.sync.dma_start(out=Cm_sb, in_=cm_d[:, :])
            nc.sync.dma_start(
                out=Wg_sb, in_=Wg_d[:, :].rearrange("(t p) n -> p t n", p=P)
            )
            # kT split by column chunk so slot g only waits on chunks <= g
            kT_r = kT_d[:, :].rearrange("(t p) n -> p t n", p=P)
            for c in range(16):
                cs = slice(c * (S // 16), (c + 1) * (S // 16))
                nc.sync.dma_start(out=kT_sb[:, :, cs], in_=kT_r[:, :, cs])
            nc.sync.dma_start(
                out=Ws2_sb, in_=Ws2_d[:, :].rearrange("(t p) n -> p t n", p=P)
            )
            # compute engines can't start at partition 4; DMA can
            nc.sync.dma_start(out=rsT[4:5, :], in_=ones[0:1, :])

            # --- phase A: pT = (W^T qT) + b, gates ---
            # psX (2 banks) stays open through B for gate + MLP psums
            psX = tc.tile_pool(name="psX", bufs=2, space="PSUM")
            psXp = psX.__enter__()
            # PE warmup while input DMAs stream: dummy matmuls so HAM
            # un-throttles / pstate ramps before the real stream starts
            with tc.tile_pool(name="warm", bufs=1, space="PSUM") as warm:
                win = const.tile([P, CHUNK], f16)
                nc.vector.memset(win, 0.0)
                wps = warm.tile([P, CHUNK], f32, tag="w")
                for _ in range(9):
                    nc.tensor.matmul(
                        wps, lhsT=win[:, 0:P], rhs=win, start=True, stop=True
                    )
            with tc.tile_pool(name="psA", bufs=3, space="PSUM") as psA:
                for n in range(ROWS // CHUNK):
                    ns = slice(n * CHUNK, (n + 1) * CHUNK)
                    for W_sb, b_sb, pT_sb in (
                        (Wp_sb, bp_sb, pTp_sb),
                        (Wh_sb, bh_sb, pTh_sb),
                    ):
                        for m in range(KT):
                            ms = slice(m * P, (m + 1) * P)
                            ps = psA.tile([P, CHUNK], f32, tag="pt")
                            for kt in range(KT):
                                nc.tensor.matmul(
                                    ps,
                                    lhsT=W_sb[:, kt, ms],
                                    rhs=qT_sb[:, kt, ns],
                                    start=(kt == 0),
                                    stop=(not with_bias and kt == KT - 1),
                                )
                            if with_bias:
                                nc.tensor.matmul(
                                    ps,
                                    lhsT=b_sb[0:1, ms],
                                    rhs=ones[0:1, ns],
                                    start=False,
                                    stop=True,
                                )
                            nc.scalar.copy(out=pT_sb[:, m, ns], in_=ps)

                # gates (needed only by MLP) after the pT stream
                for n in range(ROWS // CHUNK):
                    ns = slice(n * CHUNK, (n + 1) * CHUNK)
                    psg = psXp.tile([2, CHUNK], f32, tag="aux")
                    for kt in range(KT):
                        nc.tensor.matmul(
                            psg,
                            lhsT=Wg_sb[:, kt, :],
                            rhs=qT_sb[:, kt, ns],
                            start=(kt == 0),
                            stop=(not with_bias and kt == KT - 1),
                        )
                    if with_bias:
                        nc.tensor.matmul(
                            psg,
                            lhsT=bg_sb[0:1, :],
                            rhs=ones[0:1, ns],
                            start=False,
                            stop=True,
                        )
                    gt = fpool.tile([2, CHUNK], f16, tag="gt")
                    nc.scalar.activation(out=gt, in_=psg, func=ACT.Sigmoid)
                    nc.sync.dma_start(out=rsT[0:1, ns], in_=gt[0:1, :])
                    nc.sync.dma_start(out=rsT[2:3, ns], in_=gt[1:2, :])

            # --- phase B: causal scores + row max, MLP pipelined in 2 stages ---
            def emit_mlp_a(g):
                # h1 = relu(Ws1_aug.T @ rs)
                gs = slice(g * P, (g + 1) * P)
                ph1 = psXp.tile([P, 2, P], f32, tag="aux", name="ph1")
                for m in range(2):
                    ms = slice(m * P, (m + 1) * P)
                    nc.tensor.matmul(
                        ph1[:, m, :],
                        lhsT=Ws1_sb[0:5, ms],
                        rhs=rsT[0:5, gs],
                        start=True,
                        stop=True,
                    )
                nc.scalar.activation(out=h1T_sb[:, :, gs], in_=ph1, func=ACT.Relu)

            def emit_mlp_b(g):
                # h = h1.T @ Ws2 (+ b_s2)
                gs = slice(g * P, (g + 1) * P)
                ph = psXp.tile([P, D], f32, tag="aux", name="ph")
                for m in range(2):
                    nc.tensor.matmul(
                        ph,
                        lhsT=h1T_sb[:, m, gs],
                        rhs=Ws2_sb[:, m, :],
                        start=(m == 0),
                        stop=(not with_bias and m == 1),
                    )
                if with_bias:
                    nc.tensor.matmul(
                        ph,
                        lhsT=ones[0:1, 0:P],
                        rhs=bs2_sb[0:1, :],
                        start=False,
                        stop=True,
                    )
                ob = opool.tile([P, D], f32, tag="ob")
                nc.scalar.copy(out=ob, in_=ph)
                nc.sync.dma_start(out=out_d[gs, :], in_=ob)

            a_pending = []
            b_pending = []
            with tc.tile_pool(name="psB", bufs=6, space="PSUM") as psB:
                for g in (7, 6, 5, 4, 3, 2, 1, 0):
                    gs = slice(g * P, (g + 1) * P)
                    nch = g + 1
                    for pT_sb, ridx in ((pTp_sb, 1), (pTh_sb, 3)):
                        ngrp = nch
                        sct = fpool.tile([P, 1], f16, tag="sct")
                        scp = None
                        if ngrp > 1:
                            scp = spool.tile([P, 8], f32, tag="scp")
                        for c in range(nch):
                            cs = slice(c * CHUNK, (c + 1) * CHUNK)
                            last = c == g
                            ps = psB.tile([P, 1, CHUNK], f32, tag="sc")
                            for kt in range(KT):
                                nc.tensor.matmul(
                                    ps[:, 0, :],
                                    lhsT=pT_sb[:, kt, gs],
                                    rhs=kT_sb[:, kt, cs],
                                    start=(kt == 0),
                                    stop=(kt == KT - 1),
                                )
                            if last:
                                # strictly-causal mask for the diagonal chunk:
                                # DVE add of Cm (0 where j<i, NEG elsewhere)
                                nc.vector.tensor_tensor(
                                    out=ps[:, 0, :],
                                    in0=ps[:, 0, :],
                                    in1=Cm_sb,
                                    op=ADD,
                                )
                            red_out = sct if ngrp == 1 else scp[:, c : c + 1]
                            nc.vector.tensor_reduce(
                                out=red_out,
                                in_=ps[:, 0:1, :],
                                axis=AX.XY,
                                op=MAX,
                            )
                        if ngrp > 1:
                            nc.vector.tensor_reduce(
                                out=sct, in_=scp[:, 0:ngrp], axis=AX.X, op=MAX
                            )
                        # [128,1] -> [1,128] reorientation
                        nc.sync.dma_start(out=rsT[ridx : ridx + 1, gs], in_=sct)

                    a_pending.append(g)
                    if len(a_pending) >= 2:
                        ga = a_pending.pop(0)
                        emit_mlp_a(ga)
                        b_pending.append(ga)
                    if len(b_pending) >= 2:
                        emit_mlp_b(b_pending.pop(0))
                for ga in a_pending:
                    emit_mlp_a(ga)
                    b_pending.append(ga)
                for gb in b_pending:
                    emit_mlp_b(gb)
            psX.__exit__(None, None, None)

    nc.compile()
    return nc


def _get_program(with_bias=True):
    key = "nc" + ("_b" if with_bias else "")
    if key not in _PROGRAM_CACHE:
        _PROGRAM_CACHE[key] = _build_program(with_bias)
    return _PROGRAM_CACHE[key]


def _row_index(r):
    # global row indices (within a batch) owned by core with residue r
    return np.concatenate(
        [np.arange(P) + P * (4 * g + r) for g in range(NSLOT)]
    )


def make_in_maps(q, k, W_pre, b_pre, W_haz, b_haz, W_gate, b_gate, W_s1, b_s1,
                 W_s2, b_s2):
    """Build the 8 per-core input dicts (host-side prep)."""
    f = np.float32
    f16 = np.float16
    scale = 1.0 / np.sqrt(f(D))
    Wp = np.ascontiguousarray((W_pre * scale).astype(f).astype(f16))
    Wh = np.ascontiguousarray((W_haz * scale).astype(f).astype(f16))
    Wg = np.ascontiguousarray(W_gate.astype(f).astype(f16))
    Ws1 = np.concatenate([W_s1.astype(f), b_s1.astype(f).reshape(1, D1)], axis=0)
    Ws2 = np.ascontiguousarray(W_s2.astype(f).astype(f16))

    c = np.zeros((P, CONSTW), f)
    c[0, ONES_O : ONES_O + ROWS] = 1.0
    c[0:5, WS1_O : WS1_O + D1] = Ws1
    c[0, BP_O : BP_O + D] = (b_pre * scale).astype(f)
    c[0, BH_O : BH_O + D] = (b_haz * scale).astype(f)
    c[0, BS2_O : BS2_O + D] = b_s2.astype(f)
    c[0, BG_O : BG_O + 2] = b_gate.astype(f)
    consts = c.astype(f16)

    kTb = [np.ascontiguousarray(k[b].T.astype(f).astype(f16)) for b in range(B)]

    in_maps = []
    for core in range(NCORES):
        b, r = divmod(core, 4)
        rows = _row_index(r)
        qT = np.ascontiguousarray(q[b][rows, :].T.astype(f).astype(f16))
        pp, ff = np.mgrid[0:P, 0:CHUNK]
        cm = np.where(ff < P * r + pp, 0.0, NEG).astype(f).astype(f16)
        in_maps.append(
            {
                "qT": qT,
                "kT": kTb[b],
                "Wp": Wp,
                "Wh": Wh,
                "Wg": Wg,
                "Ws2": Ws2,
                "consts": consts,
                "cm": np.ascontiguousarray(cm),
            }
        )
    return in_maps


def assemble_output(results, q, W_gate, b_gate, W_s1, b_s1, W_s2, b_s2):
    out = np.empty((B, S, D), np.float32)
    for core in range(NCORES):
        b, r = divmod(core, 4)
        rows = _row_index(r)
        out[b][rows, :] = results[core]["out"]
    # row 0 of each batch: no visible keys -> score = 0 (exact host fixup)
    for b in range(B):
        g0 = 1.0 / (1.0 + np.exp(-(q[b, 0].astype(np.float64) @ W_gate + b_gate)))
        rs0 = np.array([g0[0], 0.0, g0[1], 0.0])
        h0 = np.maximum(rs0 @ W_s1 + b_s1, 0.0) @ W_s2 + b_s2
        out[b, 0, :] = h0.astype(np.float32)
    return out


def kernel(**inputs):
    from concourse.bass_utils import run_bass_kernel_spmd

    q = np.asarray(inputs["q"], np.float32)
    k = np.asarray(inputs["k"], np.float32)
    args = dict(
        q=q,
        k=k,
        W_pre=np.asarray(inputs["W_pre"], np.float32),
        b_pre=np.asarray(inputs["b_pre"], np.float32),
        W_haz=np.asarray(inputs["W_haz"], np.float32),
        b_haz=np.asarray(inputs["b_haz"], np.float32),
        W_gate=np.asarray(inputs["W_gate"], np.float32),
        b_gate=np.asarray(inputs["b_gate"], np.float32),
        W_s1=np.asarray(inputs["W_s1"], np.float32),
        b_s1=np.asarray(inputs["b_s1"], np.float32),
        W_s2=np.asarray(inputs["W_s2"], np.float32),
        b_s2=np.asarray(inputs["b_s2"], np.float32),
    )
    zero_bias = all(
        not np.any(args[b_]) for b_ in ("b_pre", "b_haz", "b_gate", "b_s1", "b_s2")
    )
    nc = _get_program(with_bias=not zero_bias)
    in_maps = make_in_maps(**args)
    res = run_bass_kernel_spmd(nc, in_maps, list(range(NCORES)))
    return assemble_output(
        res.results,
        q,
        args["W_gate"],
        args["b_gate"],
        args["W_s1"],
        args["b_s1"],
        args["W_s2"],
        args["b_s2"],
    )
